# revision 1
# baseline (speedup 1.0000x reference)
"""DCNNv2 GNN message-passing kernel for 8 trn2 NeuronCores — single NEFF.

Strategy (memory-regime): shard external nodes (N=10000 -> 1250/core, padded
to 1280). The E table (50000x128 f32) is replicated into each core's HBM and
all embedding-row gathers run ON DEVICE via 1-column indirect SWDGE DMAs
(multi-column offsets corrupt on this platform; the dma_gather ucode is not
shipped in this image). Neighbour sums use CCE gather-accumulate (AluOp.add
in the DMA datapath). One NEFF runs all three model phases with device-side
AllGather collectives between them:

  A: nbsum j-sum (CCE) + s=relu(W e + M t), k-sum, softmax -> h shard.
     The h AllGather is split in three chunks (nodes 0-639, 640-1151,
     1152-1279), each fired as soon as its nodes are final so the wire time
     hides under the remaining gather stream and only the last 1-block
     chunk is serially exposed; chunks land in Shared single-writer tiles
     and are merged into one Local table for the ext-neighbour gathers.
  B: ext-neighbour CCE gather-sum from the h table + relu(U h + V ext)
     + softmax -> e_all shard. Instead of AllGathering all of e_all, each
     core compacts the <=640 locally-owned rows any link-prediction pair
     touches and only those are exchanged (5 MB -> 2.6 MB).
  C: pair gathers from the compacted table + concat MLP + leaky relu +
     2-class softmax (as sigmoid of the logit difference) -> probs.

Host only reorders index tensors (int32), replicates weights, and splits the
output; no model FLOPs or data-size-bearing gathers run on host.
"""
import sys
sys.path.insert(0, "/opt/trn_rl_repo")
import numpy as np
import concourse.bacc as bacc
import concourse.mybir as mybir
import concourse.bass as bass
from concourse.tile import TileContext
from concourse.masks import make_identity
from concourse.bass_utils import run_bass_kernel_spmd

F32 = mybir.dt.float32
I32 = mybir.dt.int32
AX = mybir.AxisListType
ALU = mybir.AluOpType
ACT = mybir.ActivationFunctionType

N, K, J, D, VOC, B = 10000, 16, 8, 128, 50000, 2048
NC_ = 8
NSH = N // NC_          # 1250 real nodes per core
NS = 1280               # padded nodes per core
NT = NS * K // 128      # 160 supertiles of 128 (n,k) groups
NB = NS // 128          # 10 node blocks
NEXT = 16               # external neighbours per node
H1 = 640                # h AllGather chunk boundaries (chunks fire as soon
H2 = 1152               # as their nodes are final, overlapping phase A)
CMP = 640               # compacted e rows per core for the pair exchange


def _softmax_block(nc, pool, blk_in, out_ap):
    """softmax along free dim of a [128,128] tile; writes to out_ap (sbuf)."""
    negmax = pool.tile([128, 1], F32, tag="negmax")
    nc.vector.tensor_reduce(out=negmax[:], in_=blk_in, axis=AX.X,
                            op=ALU.max, negate=True)
    ex = pool.tile([128, 128], F32, tag="ex")
    sm = pool.tile([128, 1], F32, tag="sm")
    nc.scalar.activation(out=ex[:], in_=blk_in, func=ACT.Exp,
                         bias=negmax[:], accum_out=sm[:])
    rec = pool.tile([128, 1], F32, tag="rec")
    nc.vector.reciprocal(rec[:], sm[:])
    nc.vector.tensor_scalar_mul(out_ap, ex[:], rec[:])


def _gather(nc, dst_ap, table_ap, off_col, add=False, q=0):
    inst = nc.gpsimd.indirect_dma_start(
        out=dst_ap, out_offset=None, in_=table_ap,
        in_offset=bass.IndirectOffsetOnAxis(ap=off_col, axis=0),
        compute_op=(ALU.add if add else ALU.bypass),
    )
    if q:
        inst.ins.queue = f"qPoolDynamic{q}"
    return inst


def _h_block(nc, pool, psp, ident, R, hsb, h_shard, b):
    """transpose R block -> softmax -> h block in SBUF + DRAM shard."""
    rT_p = psp.tile([128, 128], F32, tag="rT", bufs=1)
    nc.tensor.transpose(out=rT_p[:], in_=R[:, b * 128:(b + 1) * 128],
                        identity=ident[:])
    rT = pool.tile([128, 128], F32, tag="rTs")
    nc.scalar.copy(rT[:], rT_p[:])
    _softmax_block(nc, pool, rT[:], hsb[:, b * 128:(b + 1) * 128])
    nc.sync.dma_start(out=h_shard[b * 128:(b + 1) * 128, :],
                      in_=hsb[:, b * 128:(b + 1) * 128])


def build(ncores=NC_, npairs=None, reps=1, nq=1):
    """reps>1 unrolls the whole pipeline N times (fresh DRAM tiles per rep,
    identical results) — used only for amplified timing measurements."""
    NP = npairs if npairs is not None else B // ncores   # pairs per core
    nc = bacc.Bacc("TRN2", target_bir_lowering=False, num_devices=ncores,
                   num_swdge_queues=nq)
    E = nc.dram_tensor("E", [VOC, D], F32, kind="ExternalInput")
    nbrI = nc.dram_tensor("nbrI", [128, NT * J], I32, kind="ExternalInput")
    embI = nc.dram_tensor("embI", [128, NT], I32, kind="ExternalInput")
    extI = nc.dram_tensor("extI", [128, NB * NEXT], I32, kind="ExternalInput")
    pAI = nc.dram_tensor("pAI", [128, NP // 128], I32, kind="ExternalInput")
    pBI = nc.dram_tensor("pBI", [128, NP // 128], I32, kind="ExternalInput")
    cmpI = nc.dram_tensor("cmpI", [128, CMP // 128], I32, kind="ExternalInput")
    WT = nc.dram_tensor("WT", [128, 128], F32, kind="ExternalInput")
    MT = nc.dram_tensor("MT", [128, 128], F32, kind="ExternalInput")
    UT = nc.dram_tensor("UT", [128, 128], F32, kind="ExternalInput")
    VT = nc.dram_tensor("VT", [128, 128], F32, kind="ExternalInput")
    W1aT = nc.dram_tensor("W1aT", [128, 128], F32, kind="ExternalInput")
    W1bT = nc.dram_tensor("W1bT", [128, 128], F32, kind="ExternalInput")
    b1t = nc.dram_tensor("b1t", [128, 1], F32, kind="ExternalInput")
    w2dT = nc.dram_tensor("w2dT", [128, 1], F32, kind="ExternalInput")
    b2d = nc.dram_tensor("b2d", [1, 1], F32, kind="ExternalInput")
    pout = nc.dram_tensor("pout", [2, NP], F32, kind="ExternalOutput")

    with TileContext(nc) as tc:
        with tc.tile_pool(name="w", bufs=1) as wpool, \
             tc.tile_pool(name="s", bufs=3) as pool, \
             tc.tile_pool(name="d", bufs=1, space="DRAM") as dpool:
            ident = wpool.tile([128, 128], F32)
            make_identity(nc, ident[:])
            wt = wpool.tile([128, 128], F32)
            mt = wpool.tile([128, 128], F32)
            nc.sync.dma_start(out=wt[:], in_=WT.ap())
            nc.sync.dma_start(out=mt[:], in_=MT.ap())
            nbr_i = wpool.tile([128, NT * J], I32)
            emb_i = wpool.tile([128, NT], I32)
            ext_i = wpool.tile([128, NB * NEXT], I32)
            nc.sync.dma_start(out=nbr_i[:], in_=nbrI.ap())
            nc.sync.dma_start(out=emb_i[:], in_=embI.ap())
            nc.sync.dma_start(out=ext_i[:], in_=extI.ap())
            R = wpool.tile([128, NS], F32)       # [f', node] accumulator
            hsb = wpool.tile([128, NB * 128], F32)   # h blocks [node, f]
            cmp_i = wpool.tile([128, CMP // 128], I32)
            nc.sync.dma_start(out=cmp_i[:], in_=cmpI.ap())
            ut = wpool.tile([128, 128], F32)
            vt = wpool.tile([128, 128], F32)
            nc.sync.dma_start(out=ut[:], in_=UT.ap())
            nc.sync.dma_start(out=vt[:], in_=VT.ap())
            w1a = wpool.tile([128, 128], F32)
            w1b = wpool.tile([128, 128], F32)
            b1s = wpool.tile([128, 1], F32)
            w2d = wpool.tile([128, 1], F32)
            b2s = wpool.tile([1, 1], F32)
            pa_i = wpool.tile([128, NP // 128], I32)
            pb_i = wpool.tile([128, NP // 128], I32)
            nc.sync.dma_start(out=w1a[:], in_=W1aT.ap())
            nc.sync.dma_start(out=w1b[:], in_=W1bT.ap())
            nc.sync.dma_start(out=b1s[:], in_=b1t.ap())
            nc.sync.dma_start(out=w2d[:], in_=w2dT.ap())
            nc.sync.dma_start(out=b2s[:], in_=b2d.ap())
            nc.sync.dma_start(out=pa_i[:], in_=pAI.ap())
            nc.sync.dma_start(out=pb_i[:], in_=pBI.ap())
            # one PSUM pool for all phases; tag bufs budgeted to exactly
            # 8 banks: eT(2) tT(2) acc(1) rT(1) yac(1) dl(1)
            with tc.tile_pool(name="ps", bufs=2, space="PSUM") as psp:
              for rep in range(reps):
                h_shard = dpool.tile([NS, D], F32, name=f"h_shard_{rep}")
                # three Shared AG landing zones (Shared enforces a single
                # writing instruction, and Local AG outputs are deoptimized
                # in NRT), merged into one Local table for the offset gathers
                h_full_a = dpool.tile([ncores * H1, D], F32,
                                      addr_space="Shared",
                                      name=f"h_full_a_{rep}")
                h_full_b = dpool.tile([ncores * (H2 - H1), D], F32,
                                      addr_space="Shared",
                                      name=f"h_full_b_{rep}")
                h_full_c = dpool.tile([ncores * (NS - H2), D], F32,
                                      addr_space="Shared",
                                      name=f"h_full_c_{rep}")
                h_full = dpool.tile([ncores * NS, D], F32,
                                    name=f"h_full_{rep}")
                e_shard = dpool.tile([NS, D], F32, name=f"e_shard_{rep}")
                ec_shard = dpool.tile([CMP, D], F32, name=f"ec_shard_{rep}")
                ec_full = dpool.tile([ncores * CMP, D], F32,
                                     addr_space="Shared",
                                     name=f"ec_full_{rep}")

                # ------------ Phase A: internal graph conv -> h shard ------
                for t in range(NT):
                    nb = pool.tile([128, 128], F32, tag="nb", bufs=8)
                    for j in range(J):
                        _gather(nc, nb[:], E[:], nbr_i[:, t * J + j:t * J + j + 1],
                                add=(j > 0), q=t % nq)
                    emb = pool.tile([128, 128], F32, tag="emb", bufs=8)
                    _gather(nc, emb[:], E[:], emb_i[:, t:t + 1], q=t % nq)
                    eT_p = psp.tile([128, 128], F32, tag="eT")
                    nc.tensor.transpose(out=eT_p[:], in_=emb[:], identity=ident[:])
                    eT = pool.tile([128, 128], F32, tag="eTs")
                    nc.scalar.copy(eT[:], eT_p[:])
                    tT_p = psp.tile([128, 128], F32, tag="tT")
                    nc.tensor.transpose(out=tT_p[:], in_=nb[:], identity=ident[:])
                    tT = pool.tile([128, 128], F32, tag="tTs")
                    nc.scalar.copy(tT[:], tT_p[:])
                    acc = psp.tile([128, 128], F32, tag="acc", bufs=1)
                    nc.tensor.matmul(out=acc[:], lhsT=wt[:], rhs=eT[:],
                                     start=True, stop=False)
                    nc.tensor.matmul(out=acc[:], lhsT=mt[:], rhs=tT[:],
                                     start=False, stop=True)
                    s = pool.tile([128, 128], F32, tag="s")
                    nc.scalar.activation(out=s[:], in_=acc[:], func=ACT.Relu)
                    # k-sum: 8 nodes x 16 k per supertile -> [128, 8]
                    k8 = pool.tile([128, 8 * 8], F32, tag="k8")
                    sv = s[:].rearrange("p (n k) -> p n k", k=16)
                    nc.vector.tensor_tensor(
                        out=k8[:].rearrange("p (n k) -> p n k", k=8),
                        in0=sv[:, :, 0:8], in1=sv[:, :, 8:16], op=ALU.add)
                    k4 = pool.tile([128, 8 * 4], F32, tag="k4")
                    k8v = k8[:].rearrange("p (n k) -> p n k", k=8)
                    nc.vector.tensor_tensor(
                        out=k4[:].rearrange("p (n k) -> p n k", k=4),
                        in0=k8v[:, :, 0:4], in1=k8v[:, :, 4:8], op=ALU.add)
                    k2 = pool.tile([128, 8 * 2], F32, tag="k2")
                    k4v = k4[:].rearrange("p (n k) -> p n k", k=4)
                    nc.vector.tensor_tensor(
                        out=k2[:].rearrange("p (n k) -> p n k", k=2),
                        in0=k4v[:, :, 0:2], in1=k4v[:, :, 2:4], op=ALU.add)
                    k2v = k2[:].rearrange("p (n k) -> p n k", k=2)
                    nc.vector.tensor_tensor(
                        out=R[:, t * 8:(t + 1) * 8],
                        in0=k2v[:, :, 0:1].rearrange("p n k -> p (n k)"),
                        in1=k2v[:, :, 1:2].rearrange("p n k -> p (n k)"),
                        op=ALU.add)
                    if t == H1 * K // 128 - 1:
                        # nodes [0, H1) final: softmax + fire the first AG
                        # chunk so it overlaps the remaining gather stream
                        for b in range(H1 // 128):
                            _h_block(nc, pool, psp, ident, R, hsb, h_shard, b)
                        nc.gpsimd.collective_compute(
                            "AllGather", ALU.bypass,
                            replica_groups=[list(range(ncores))],
                            ins=[h_shard[0:H1, :]],
                            outs=[h_full_a[:]],
                        )
                        nc.sync.dma_start(out=h_full[0:ncores * H1, :],
                                          in_=h_full_a[:])
                    if t == H2 * K // 128 - 1:
                        # nodes [H1, H2) final: second AG chunk
                        for b in range(H1 // 128, H2 // 128):
                            _h_block(nc, pool, psp, ident, R, hsb, h_shard, b)
                        nc.gpsimd.collective_compute(
                            "AllGather", ALU.bypass,
                            replica_groups=[list(range(ncores))],
                            ins=[h_shard[H1:H2, :]],
                            outs=[h_full_b[:]],
                        )
                        nc.sync.dma_start(
                            out=h_full[ncores * H1:ncores * H2, :],
                            in_=h_full_b[:])
                # last chunk: only nodes [H2, NS) remain
                for b in range(H2 // 128, NB):
                    _h_block(nc, pool, psp, ident, R, hsb, h_shard, b)

                nc.gpsimd.collective_compute(
                    "AllGather", ALU.bypass,
                    replica_groups=[list(range(ncores))],
                    ins=[h_shard[H2:NS, :]],
                    outs=[h_full_c[:]],
                )
                nc.sync.dma_start(out=h_full[ncores * H2:ncores * NS, :],
                                  in_=h_full_c[:])

                # -------------- Phase B: external graph conv -> e_all ------
                for b in range(NB):
                    es = pool.tile([128, 128], F32, tag="es", bufs=4)
                    for j in range(NEXT):
                        _gather(nc, es[:], h_full[:],
                                ext_i[:, b * NEXT + j:b * NEXT + j + 1],
                                add=(j > 0), q=b % nq)
                    hT_p = psp.tile([128, 128], F32, tag="eT")
                    nc.tensor.transpose(out=hT_p[:],
                                        in_=hsb[:, b * 128:(b + 1) * 128],
                                        identity=ident[:])
                    hT = pool.tile([128, 128], F32, tag="eTs")
                    nc.scalar.copy(hT[:], hT_p[:])
                    xT_p = psp.tile([128, 128], F32, tag="tT")
                    nc.tensor.transpose(out=xT_p[:], in_=es[:], identity=ident[:])
                    xT = pool.tile([128, 128], F32, tag="tTs")
                    nc.scalar.copy(xT[:], xT_p[:])
                    acc2 = psp.tile([128, 128], F32, tag="acc", bufs=1)
                    nc.tensor.matmul(out=acc2[:], lhsT=ut[:], rhs=hT[:],
                                     start=True, stop=False)
                    nc.tensor.matmul(out=acc2[:], lhsT=vt[:], rhs=xT[:],
                                     start=False, stop=True)
                    pre = pool.tile([128, 128], F32, tag="s")
                    nc.scalar.activation(out=pre[:], in_=acc2[:], func=ACT.Relu)
                    pT_p = psp.tile([128, 128], F32, tag="rT", bufs=1)
                    nc.tensor.transpose(out=pT_p[:], in_=pre[:], identity=ident[:])
                    pT = pool.tile([128, 128], F32, tag="rTs")
                    nc.scalar.copy(pT[:], pT_p[:])
                    eblk = pool.tile([128, 128], F32, tag="eblk")
                    _softmax_block(nc, pool, pT[:], eblk[:])
                    nc.sync.dma_start(out=e_shard[b * 128:(b + 1) * 128, :],
                                      in_=eblk[:])

                # compact to the <=CMP locally-owned rows any pair needs,
                # then exchange only those
                for g in range(CMP // 128):
                    ct = pool.tile([128, 128], F32, tag="ct")
                    _gather(nc, ct[:], e_shard[:], cmp_i[:, g:g + 1])
                    nc.sync.dma_start(out=ec_shard[g * 128:(g + 1) * 128, :],
                                      in_=ct[:])
                nc.gpsimd.collective_compute(
                    "AllGather", ALU.bypass,
                    replica_groups=[list(range(ncores))],
                    ins=[ec_shard[:]], outs=[ec_full[:]],
                )

                # ------------ Phase C: link MLP ----------------------------
                for ch in range(NP // 256):
                    yac = psp.tile([128, 256], F32, tag="yac", bufs=1)
                    for half in range(2):
                        hh = ch * 2 + half
                        ea = pool.tile([128, 128], F32, tag="ea")
                        _gather(nc, ea[:], ec_full[:], pa_i[:, hh:hh + 1])
                        eT_p = psp.tile([128, 128], F32, tag="eT")
                        nc.tensor.transpose(out=eT_p[:], in_=ea[:],
                                            identity=ident[:])
                        eT = pool.tile([128, 128], F32, tag="eTs")
                        nc.scalar.copy(eT[:], eT_p[:])
                        nc.tensor.matmul(out=yac[:, half * 128:(half + 1) * 128],
                                         lhsT=w1a[:], rhs=eT[:],
                                         start=True, stop=False)
                        eb = pool.tile([128, 128], F32, tag="eb")
                        _gather(nc, eb[:], ec_full[:], pb_i[:, hh:hh + 1])
                        bT_p = psp.tile([128, 128], F32, tag="tT")
                        nc.tensor.transpose(out=bT_p[:], in_=eb[:],
                                            identity=ident[:])
                        bT = pool.tile([128, 128], F32, tag="tTs")
                        nc.scalar.copy(bT[:], bT_p[:])
                        nc.tensor.matmul(out=yac[:, half * 128:(half + 1) * 128],
                                         lhsT=w1b[:], rhs=bT[:],
                                         start=False, stop=True)
                    y0 = pool.tile([128, 256], F32, tag="y0")
                    nc.scalar.activation(out=y0[:], in_=yac[:], func=ACT.Identity,
                                         bias=b1s[:])
                    ys = pool.tile([128, 256], F32, tag="ys")
                    nc.scalar.mul(ys[:], y0[:], 0.01)
                    y = pool.tile([128, 256], F32, tag="y")
                    nc.vector.tensor_tensor(out=y[:], in0=y0[:], in1=ys[:],
                                            op=ALU.max)
                    dl = psp.tile([1, 256], F32, tag="dl", bufs=1)
                    nc.tensor.matmul(out=dl[:], lhsT=w2d[:, 0:1], rhs=y[:],
                                     start=True, stop=True)
                    p0 = pool.tile([1, 256], F32, tag="p0")
                    nc.scalar.activation(out=p0[:], in_=dl[:], func=ACT.Sigmoid,
                                         bias=b2s[:], scale=1.0)
                    nb2 = pool.tile([1, 1], F32, tag="nb2")
                    nc.scalar.mul(nb2[:], b2s[:], -1.0)
                    p1 = pool.tile([1, 256], F32, tag="p1")
                    nc.scalar.activation(out=p1[:], in_=dl[:], func=ACT.Sigmoid,
                                         bias=nb2[:], scale=-1.0)
                    nc.sync.dma_start(out=pout[0:1, ch * 256:(ch + 1) * 256],
                                      in_=p0[:])
                    nc.sync.dma_start(out=pout[1:2, ch * 256:(ch + 1) * 256],
                                      in_=p1[:])
    nc.compile()
    return nc


def preprocess(batch, int_node_ids, int_neigh_ids, ext_neigh,
               E, W, M, U, V, W1, b1, W2, b2, ncores=NC_):
    """Build per-core input maps (index reordering + weight replication)."""
    nsh = N // ncores
    NP = B // ncores
    ids = np.asarray(int_node_ids).astype(np.int32)
    idn = np.asarray(int_neigh_ids).astype(np.int32)
    ext = np.asarray(ext_neigh).astype(np.int64)
    bat = np.asarray(batch).astype(np.int64)
    # h_full row for global node n (3-chunk split AllGather: each chunk's
    # per-core slices land consecutively)
    own_e = ext // nsh
    loc_e = ext % nsh
    extg = np.where(
        loc_e < H1, own_e * H1 + loc_e,
        np.where(loc_e < H2,
                 ncores * H1 + own_e * (H2 - H1) + (loc_e - H1),
                 ncores * H2 + own_e * (NS - H2) + (loc_e - H2))
    ).astype(np.int32)
    # ec_full slot for each pair endpoint: per-owner compacted unique rows
    allp = np.concatenate([bat[:, 0], bat[:, 1]])
    slot_of = np.full(N, -1, np.int64)
    cmp_lists = []
    for c in range(ncores):
        u = np.unique(allp[allp // nsh == c] % nsh)
        if len(u) > CMP:
            raise ValueError(f"core {c} needs {len(u)} > {CMP} pair rows")
        slot_of[c * nsh + u] = c * CMP + np.arange(len(u))
        lu = np.zeros(CMP, np.int32)
        lu[:len(u)] = u
        cmp_lists.append(lu)
    bga = slot_of[bat[:, 0]].astype(np.int32)
    bgb = slot_of[bat[:, 1]].astype(np.int32)
    assert bga.min() >= 0 and bgb.min() >= 0

    Ef = np.ascontiguousarray(np.asarray(E, np.float32))
    w = {
        "WT": np.ascontiguousarray(np.asarray(W, np.float32).T),
        "MT": np.ascontiguousarray(np.asarray(M, np.float32).T),
        "UT": np.ascontiguousarray(np.asarray(U, np.float32).T),
        "VT": np.ascontiguousarray(np.asarray(V, np.float32).T),
        "W1aT": np.ascontiguousarray(np.asarray(W1, np.float32)[:, :128].T),
        "W1bT": np.ascontiguousarray(np.asarray(W1, np.float32)[:, 128:].T),
        "b1t": np.asarray(b1, np.float32).reshape(128, 1),
        "w2dT": np.ascontiguousarray(
            (np.asarray(W2, np.float32)[0] - np.asarray(W2, np.float32)[1])
            .reshape(128, 1)),
        "b2d": np.array([[np.float32(b2[0]) - np.float32(b2[1])]], np.float32),
    }
    in_maps = []
    for c in range(ncores):
        lo = c * nsh
        ids_p = np.zeros((NS, K), np.int32)
        idn_p = np.zeros((NS, K, J), np.int32)
        ext_p = np.zeros((NS, NEXT), np.int32)
        ids_p[:nsh] = ids[lo:lo + nsh]
        idn_p[:nsh] = idn[lo:lo + nsh]
        ext_p[:nsh] = extg[lo:lo + nsh]
        embI = np.ascontiguousarray(ids_p.reshape(NT, 128).T)
        nbrI = np.ascontiguousarray(
            idn_p.reshape(NT, 128, J).transpose(1, 0, 2).reshape(128, NT * J))
        extI = np.ascontiguousarray(
            ext_p.reshape(NB, 128, NEXT).transpose(1, 0, 2)
            .reshape(128, NB * NEXT))
        pAIc = np.ascontiguousarray(
            bga[c * NP:(c + 1) * NP].reshape(NP // 128, 128).T)
        pBIc = np.ascontiguousarray(
            bgb[c * NP:(c + 1) * NP].reshape(NP // 128, 128).T)
        cmpIc = np.ascontiguousarray(
            cmp_lists[c].reshape(CMP // 128, 128).T)
        in_maps.append({"E": Ef, "nbrI": nbrI, "embI": embI, "extI": extI,
                        "pAI": pAIc, "pBI": pBIc, "cmpI": cmpIc, **w})
    return in_maps


def postprocess(results, ncores=NC_):
    NP = B // ncores
    out = np.zeros((B, 2), np.float32)
    for c in range(ncores):
        p = results[c]["pout"]              # [2, NP]
        out[c * NP:(c + 1) * NP, 0] = p[0]
        out[c * NP:(c + 1) * NP, 1] = p[1]
    return out


_NC_CACHE = None


def kernel(batch, int_node_ids, int_neigh_ids, ext_neigh,
           E, W, M, U, V, W1, b1, W2, b2):
    global _NC_CACHE
    in_maps = preprocess(batch, int_node_ids, int_neigh_ids, ext_neigh,
                         E, W, M, U, V, W1, b1, W2, b2)
    if _NC_CACHE is None:
        _NC_CACHE = build()
    res = run_bass_kernel_spmd(_NC_CACHE, in_maps, core_ids=list(range(NC_)))
    return postprocess(res.results)



# revision 3
# speedup vs baseline: 1.3383x; 1.3383x over previous
"""DCNNv2 GNN message-passing kernel for 8 trn2 NeuronCores — single NEFF.

Strategy (memory-regime): shard external nodes (N=10000 -> 1250/core, padded
to 1280). The E table (50000x128 f32) is replicated into each core's HBM and
all embedding-row gathers run ON DEVICE via 1-column indirect SWDGE DMAs
(multi-column offsets corrupt on this platform; the dma_gather ucode is not
shipped in this image). Neighbour sums use CCE gather-accumulate (AluOp.add
in the DMA datapath). One NEFF runs all three model phases with device-side
AllGather collectives between them:

  A: nbsum j-sum (CCE) + s=relu(W e + M t), k-sum, softmax -> h shard.
     The h AllGather is split in three chunks (nodes 0-639, 640-1151,
     1152-1279), each fired as soon as its nodes are final so the wire time
     hides under the remaining gather stream and only the last 1-block
     chunk is serially exposed; chunks land in Shared single-writer tiles
     and are merged into one Local table for the ext-neighbour gathers.
  B: ext-neighbour CCE gather-sum from the h table + relu(U h + V ext)
     + softmax -> e_all shard. Instead of AllGathering all of e_all, each
     core compacts the <=640 locally-owned rows any link-prediction pair
     touches and only those are exchanged (5 MB -> 2.6 MB).
  C: pair gathers from the compacted table + concat MLP + leaky relu +
     2-class softmax (as sigmoid of the logit difference) -> probs.

Host only reorders index tensors (int32), replicates weights, and splits the
output; no model FLOPs or data-size-bearing gathers run on host.
"""
import sys
sys.path.insert(0, "/opt/trn_rl_repo")
import numpy as np
import concourse.bacc as bacc
import concourse.mybir as mybir
import concourse.bass as bass
from concourse.tile import TileContext
from concourse.masks import make_identity
from concourse.bass_utils import run_bass_kernel_spmd

F32 = mybir.dt.float32
I32 = mybir.dt.int32
AX = mybir.AxisListType
ALU = mybir.AluOpType
ACT = mybir.ActivationFunctionType

N, K, J, D, VOC, B = 10000, 16, 8, 128, 50000, 2048
NC_ = 8
NSH = N // NC_          # 1250 real nodes per core
NS = 1280               # padded nodes per core
NT = NS * K // 128      # 160 supertiles of 128 (n,k) groups
NB = NS // 128          # 10 node blocks
NEXT = 16               # external neighbours per node
H1 = 640                # h AllGather chunk boundaries (chunks fire as soon
H2 = 1152               # as their nodes are final, overlapping phase A)
CMP = 640               # compacted e rows per core for the pair exchange


def _softmax_block(nc, pool, blk_in, out_ap):
    """softmax along free dim of a [128,128] tile; writes to out_ap (sbuf)."""
    negmax = pool.tile([128, 1], F32, tag="negmax")
    nc.vector.tensor_reduce(out=negmax[:], in_=blk_in, axis=AX.X,
                            op=ALU.max, negate=True)
    ex = pool.tile([128, 128], F32, tag="ex")
    sm = pool.tile([128, 1], F32, tag="sm")
    nc.scalar.activation(out=ex[:], in_=blk_in, func=ACT.Exp,
                         bias=negmax[:], accum_out=sm[:])
    rec = pool.tile([128, 1], F32, tag="rec")
    nc.vector.reciprocal(rec[:], sm[:])
    nc.vector.tensor_scalar_mul(out_ap, ex[:], rec[:])


def _gather(nc, dst_ap, table_ap, off_col, add=False, q=0):
    inst = nc.gpsimd.indirect_dma_start(
        out=dst_ap, out_offset=None, in_=table_ap,
        in_offset=bass.IndirectOffsetOnAxis(ap=off_col, axis=0),
        compute_op=(ALU.add if add else ALU.bypass),
    )
    if q:
        inst.ins.queue = f"qPoolDynamic{q}"
    return inst


def _h_block(nc, pool, psp, ident, R, hsb, h_shard, b):
    """transpose R block -> softmax -> h block in SBUF + DRAM shard."""
    rT_p = psp.tile([128, 128], F32, tag="rT", bufs=1)
    nc.tensor.transpose(out=rT_p[:], in_=R[:, b * 128:(b + 1) * 128],
                        identity=ident[:])
    rT = pool.tile([128, 128], F32, tag="rTs")
    nc.scalar.copy(rT[:], rT_p[:])
    _softmax_block(nc, pool, rT[:], hsb[:, b * 128:(b + 1) * 128])
    nc.sync.dma_start(out=h_shard[b * 128:(b + 1) * 128, :],
                      in_=hsb[:, b * 128:(b + 1) * 128])


def build(ncores=NC_, npairs=None, reps=1, nq=1):
    """reps>1 unrolls the whole pipeline N times (fresh DRAM tiles per rep,
    identical results) — used only for amplified timing measurements."""
    NP = npairs if npairs is not None else B // ncores   # pairs per core
    nc = bacc.Bacc("TRN2", target_bir_lowering=False, num_devices=ncores,
                   num_swdge_queues=nq)
    E = nc.dram_tensor("E", [VOC, D], F32, kind="ExternalInput")
    nbrI = nc.dram_tensor("nbrI", [128, NT * J], I32, kind="ExternalInput")
    embI = nc.dram_tensor("embI", [128, NT], I32, kind="ExternalInput")
    extI = nc.dram_tensor("extI", [128, NB * NEXT], I32, kind="ExternalInput")
    pAI = nc.dram_tensor("pAI", [128, NP // 128], I32, kind="ExternalInput")
    pBI = nc.dram_tensor("pBI", [128, NP // 128], I32, kind="ExternalInput")
    cmpI = nc.dram_tensor("cmpI", [128, CMP // 128], I32, kind="ExternalInput")
    WT = nc.dram_tensor("WT", [128, 128], F32, kind="ExternalInput")
    MT = nc.dram_tensor("MT", [128, 128], F32, kind="ExternalInput")
    UT = nc.dram_tensor("UT", [128, 128], F32, kind="ExternalInput")
    VT = nc.dram_tensor("VT", [128, 128], F32, kind="ExternalInput")
    W1aT = nc.dram_tensor("W1aT", [128, 128], F32, kind="ExternalInput")
    W1bT = nc.dram_tensor("W1bT", [128, 128], F32, kind="ExternalInput")
    b1t = nc.dram_tensor("b1t", [128, 1], F32, kind="ExternalInput")
    w2dT = nc.dram_tensor("w2dT", [128, 1], F32, kind="ExternalInput")
    b2d = nc.dram_tensor("b2d", [1, 1], F32, kind="ExternalInput")
    pout = nc.dram_tensor("pout", [2, NP], F32, kind="ExternalOutput")

    with TileContext(nc) as tc:
        with tc.tile_pool(name="w", bufs=1) as wpool, \
             tc.tile_pool(name="s", bufs=3) as pool, \
             tc.tile_pool(name="d", bufs=1, space="DRAM") as dpool:
            ident = wpool.tile([128, 128], F32)
            make_identity(nc, ident[:])
            wt = wpool.tile([128, 128], F32)
            mt = wpool.tile([128, 128], F32)
            nc.sync.dma_start(out=wt[:], in_=WT.ap())
            nc.sync.dma_start(out=mt[:], in_=MT.ap())
            nbr_i = wpool.tile([128, NT * J], I32)
            emb_i = wpool.tile([128, NT], I32)
            ext_i = wpool.tile([128, NB * NEXT], I32)
            nc.sync.dma_start(out=nbr_i[:], in_=nbrI.ap())
            nc.sync.dma_start(out=emb_i[:], in_=embI.ap())
            nc.sync.dma_start(out=ext_i[:], in_=extI.ap())
            R = wpool.tile([128, NS], F32)       # [f', node] accumulator
            hsb = wpool.tile([128, NB * 128], F32)   # h blocks [node, f]
            cmp_i = wpool.tile([128, CMP // 128], I32)
            nc.sync.dma_start(out=cmp_i[:], in_=cmpI.ap())
            ut = wpool.tile([128, 128], F32)
            vt = wpool.tile([128, 128], F32)
            nc.sync.dma_start(out=ut[:], in_=UT.ap())
            nc.sync.dma_start(out=vt[:], in_=VT.ap())
            w1a = wpool.tile([128, 128], F32)
            w1b = wpool.tile([128, 128], F32)
            b1s = wpool.tile([128, 1], F32)
            w2d = wpool.tile([128, 1], F32)
            b2s = wpool.tile([1, 1], F32)
            pa_i = wpool.tile([128, NP // 128], I32)
            pb_i = wpool.tile([128, NP // 128], I32)
            nc.sync.dma_start(out=w1a[:], in_=W1aT.ap())
            nc.sync.dma_start(out=w1b[:], in_=W1bT.ap())
            nc.sync.dma_start(out=b1s[:], in_=b1t.ap())
            nc.sync.dma_start(out=w2d[:], in_=w2dT.ap())
            nc.sync.dma_start(out=b2s[:], in_=b2d.ap())
            nc.sync.dma_start(out=pa_i[:], in_=pAI.ap())
            nc.sync.dma_start(out=pb_i[:], in_=pBI.ap())
            # one PSUM pool for all phases; tag bufs budgeted to exactly
            # 8 banks: eT(2) tT(2) acc(1) rT(1) yac(1) dl(1)
            with tc.tile_pool(name="ps", bufs=2, space="PSUM") as psp:
              for rep in range(reps):
                h_shard = dpool.tile([NS, D], F32, name=f"h_shard_{rep}")
                # three Shared AG landing zones (Shared enforces a single
                # writing instruction, and Local AG outputs are deoptimized
                # in NRT), merged into one Local table for the offset gathers
                h_full_a = dpool.tile([ncores * H1, D], F32,
                                      addr_space="Shared",
                                      name=f"h_full_a_{rep}")
                h_full_b = dpool.tile([ncores * (H2 - H1), D], F32,
                                      addr_space="Shared",
                                      name=f"h_full_b_{rep}")
                h_full_c = dpool.tile([ncores * (NS - H2), D], F32,
                                      addr_space="Shared",
                                      name=f"h_full_c_{rep}")
                h_full = dpool.tile([ncores * NS, D], F32,
                                    name=f"h_full_{rep}")
                e_shard = dpool.tile([NS, D], F32, name=f"e_shard_{rep}")
                ec_shard = dpool.tile([CMP, D], F32, name=f"ec_shard_{rep}")
                ec_full = dpool.tile([ncores * CMP, D], F32,
                                     addr_space="Shared",
                                     name=f"ec_full_{rep}")

                # ------------ Phase A: internal graph conv -> h shard ------
                # The 8 neighbour gathers land in DISJOINT slices of one wide
                # tile (no WAW deps -> the Pool sequencer streams them at
                # line rate; accumulate-in-place CCE chains would serialize
                # at ~2us per dependent gather). The j-sum runs as a 3-level
                # contiguous add tree on the Vector engine, hidden under the
                # gather stream.
                for t in range(NT):
                    nb8 = pool.tile([128, J * 128], F32, tag="nb8", bufs=6)
                    for j in range(J):
                        _gather(nc, nb8[:, j * 128:(j + 1) * 128], E[:],
                                nbr_i[:, t * J + j:t * J + j + 1], q=t % nq)
                    emb = pool.tile([128, 128], F32, tag="emb", bufs=8)
                    _gather(nc, emb[:], E[:], emb_i[:, t:t + 1], q=t % nq)
                    t512 = pool.tile([128, 512], F32, tag="t512")
                    nc.vector.tensor_tensor(out=t512[:], in0=nb8[:, 0:512],
                                            in1=nb8[:, 512:1024], op=ALU.add)
                    t256 = pool.tile([128, 256], F32, tag="t256")
                    nc.vector.tensor_tensor(out=t256[:], in0=t512[:, 0:256],
                                            in1=t512[:, 256:512], op=ALU.add)
                    nbs = pool.tile([128, 128], F32, tag="nbs")
                    nc.vector.tensor_tensor(out=nbs[:], in0=t256[:, 0:128],
                                            in1=t256[:, 128:256], op=ALU.add)
                    eT_p = psp.tile([128, 128], F32, tag="eT")
                    nc.tensor.transpose(out=eT_p[:], in_=emb[:], identity=ident[:])
                    eT = pool.tile([128, 128], F32, tag="eTs")
                    nc.scalar.copy(eT[:], eT_p[:])
                    tT_p = psp.tile([128, 128], F32, tag="tT")
                    nc.tensor.transpose(out=tT_p[:], in_=nbs[:], identity=ident[:])
                    tT = pool.tile([128, 128], F32, tag="tTs")
                    nc.scalar.copy(tT[:], tT_p[:])
                    acc = psp.tile([128, 128], F32, tag="acc", bufs=1)
                    nc.tensor.matmul(out=acc[:], lhsT=wt[:], rhs=eT[:],
                                     start=True, stop=False)
                    nc.tensor.matmul(out=acc[:], lhsT=mt[:], rhs=tT[:],
                                     start=False, stop=True)
                    s = pool.tile([128, 128], F32, tag="s")
                    nc.scalar.activation(out=s[:], in_=acc[:], func=ACT.Relu)
                    # k-sum: 8 nodes x 16 k per supertile -> [128, 8]
                    k8 = pool.tile([128, 8 * 8], F32, tag="k8")
                    sv = s[:].rearrange("p (n k) -> p n k", k=16)
                    nc.vector.tensor_tensor(
                        out=k8[:].rearrange("p (n k) -> p n k", k=8),
                        in0=sv[:, :, 0:8], in1=sv[:, :, 8:16], op=ALU.add)
                    k4 = pool.tile([128, 8 * 4], F32, tag="k4")
                    k8v = k8[:].rearrange("p (n k) -> p n k", k=8)
                    nc.vector.tensor_tensor(
                        out=k4[:].rearrange("p (n k) -> p n k", k=4),
                        in0=k8v[:, :, 0:4], in1=k8v[:, :, 4:8], op=ALU.add)
                    k2 = pool.tile([128, 8 * 2], F32, tag="k2")
                    k4v = k4[:].rearrange("p (n k) -> p n k", k=4)
                    nc.vector.tensor_tensor(
                        out=k2[:].rearrange("p (n k) -> p n k", k=2),
                        in0=k4v[:, :, 0:2], in1=k4v[:, :, 2:4], op=ALU.add)
                    k2v = k2[:].rearrange("p (n k) -> p n k", k=2)
                    nc.vector.tensor_tensor(
                        out=R[:, t * 8:(t + 1) * 8],
                        in0=k2v[:, :, 0:1].rearrange("p n k -> p (n k)"),
                        in1=k2v[:, :, 1:2].rearrange("p n k -> p (n k)"),
                        op=ALU.add)
                    if t == H1 * K // 128 - 1:
                        # nodes [0, H1) final: softmax + fire the first AG
                        # chunk so it overlaps the remaining gather stream
                        for b in range(H1 // 128):
                            _h_block(nc, pool, psp, ident, R, hsb, h_shard, b)
                        nc.gpsimd.collective_compute(
                            "AllGather", ALU.bypass,
                            replica_groups=[list(range(ncores))],
                            ins=[h_shard[0:H1, :]],
                            outs=[h_full_a[:]],
                        )
                        nc.sync.dma_start(out=h_full[0:ncores * H1, :],
                                          in_=h_full_a[:])
                    if t == H2 * K // 128 - 1:
                        # nodes [H1, H2) final: second AG chunk
                        for b in range(H1 // 128, H2 // 128):
                            _h_block(nc, pool, psp, ident, R, hsb, h_shard, b)
                        nc.gpsimd.collective_compute(
                            "AllGather", ALU.bypass,
                            replica_groups=[list(range(ncores))],
                            ins=[h_shard[H1:H2, :]],
                            outs=[h_full_b[:]],
                        )
                        nc.sync.dma_start(
                            out=h_full[ncores * H1:ncores * H2, :],
                            in_=h_full_b[:])
                # last chunk: only nodes [H2, NS) remain
                for b in range(H2 // 128, NB):
                    _h_block(nc, pool, psp, ident, R, hsb, h_shard, b)

                nc.gpsimd.collective_compute(
                    "AllGather", ALU.bypass,
                    replica_groups=[list(range(ncores))],
                    ins=[h_shard[H2:NS, :]],
                    outs=[h_full_c[:]],
                )
                nc.sync.dma_start(out=h_full[ncores * H2:ncores * NS, :],
                                  in_=h_full_c[:])

                # -------------- Phase B: external graph conv -> e_all ------
                for b in range(NB):
                    es16 = pool.tile([128, NEXT * 128], F32, tag="es16", bufs=3)
                    for j in range(NEXT):
                        _gather(nc, es16[:, j * 128:(j + 1) * 128], h_full[:],
                                ext_i[:, b * NEXT + j:b * NEXT + j + 1],
                                q=b % nq)
                    u1 = pool.tile([128, 1024], F32, tag="u1")
                    nc.vector.tensor_tensor(out=u1[:], in0=es16[:, 0:1024],
                                            in1=es16[:, 1024:2048], op=ALU.add)
                    u2 = pool.tile([128, 512], F32, tag="u2")
                    nc.vector.tensor_tensor(out=u2[:], in0=u1[:, 0:512],
                                            in1=u1[:, 512:1024], op=ALU.add)
                    u3 = pool.tile([128, 256], F32, tag="u3")
                    nc.vector.tensor_tensor(out=u3[:], in0=u2[:, 0:256],
                                            in1=u2[:, 256:512], op=ALU.add)
                    es = pool.tile([128, 128], F32, tag="es")
                    nc.vector.tensor_tensor(out=es[:], in0=u3[:, 0:128],
                                            in1=u3[:, 128:256], op=ALU.add)
                    hT_p = psp.tile([128, 128], F32, tag="eT")
                    nc.tensor.transpose(out=hT_p[:],
                                        in_=hsb[:, b * 128:(b + 1) * 128],
                                        identity=ident[:])
                    hT = pool.tile([128, 128], F32, tag="eTs")
                    nc.scalar.copy(hT[:], hT_p[:])
                    xT_p = psp.tile([128, 128], F32, tag="tT")
                    nc.tensor.transpose(out=xT_p[:], in_=es[:], identity=ident[:])
                    xT = pool.tile([128, 128], F32, tag="tTs")
                    nc.scalar.copy(xT[:], xT_p[:])
                    acc2 = psp.tile([128, 128], F32, tag="acc", bufs=1)
                    nc.tensor.matmul(out=acc2[:], lhsT=ut[:], rhs=hT[:],
                                     start=True, stop=False)
                    nc.tensor.matmul(out=acc2[:], lhsT=vt[:], rhs=xT[:],
                                     start=False, stop=True)
                    pre = pool.tile([128, 128], F32, tag="s")
                    nc.scalar.activation(out=pre[:], in_=acc2[:], func=ACT.Relu)
                    pT_p = psp.tile([128, 128], F32, tag="rT", bufs=1)
                    nc.tensor.transpose(out=pT_p[:], in_=pre[:], identity=ident[:])
                    pT = pool.tile([128, 128], F32, tag="rTs")
                    nc.scalar.copy(pT[:], pT_p[:])
                    eblk = pool.tile([128, 128], F32, tag="eblk")
                    _softmax_block(nc, pool, pT[:], eblk[:])
                    nc.sync.dma_start(out=e_shard[b * 128:(b + 1) * 128, :],
                                      in_=eblk[:])

                # compact to the <=CMP locally-owned rows any pair needs,
                # then exchange only those
                for g in range(CMP // 128):
                    ct = pool.tile([128, 128], F32, tag="ct")
                    _gather(nc, ct[:], e_shard[:], cmp_i[:, g:g + 1])
                    nc.sync.dma_start(out=ec_shard[g * 128:(g + 1) * 128, :],
                                      in_=ct[:])
                nc.gpsimd.collective_compute(
                    "AllGather", ALU.bypass,
                    replica_groups=[list(range(ncores))],
                    ins=[ec_shard[:]], outs=[ec_full[:]],
                )

                # ------------ Phase C: link MLP ----------------------------
                for ch in range(NP // 256):
                    yac = psp.tile([128, 256], F32, tag="yac", bufs=1)
                    for half in range(2):
                        hh = ch * 2 + half
                        ea = pool.tile([128, 128], F32, tag="ea")
                        _gather(nc, ea[:], ec_full[:], pa_i[:, hh:hh + 1])
                        eT_p = psp.tile([128, 128], F32, tag="eT")
                        nc.tensor.transpose(out=eT_p[:], in_=ea[:],
                                            identity=ident[:])
                        eT = pool.tile([128, 128], F32, tag="eTs")
                        nc.scalar.copy(eT[:], eT_p[:])
                        nc.tensor.matmul(out=yac[:, half * 128:(half + 1) * 128],
                                         lhsT=w1a[:], rhs=eT[:],
                                         start=True, stop=False)
                        eb = pool.tile([128, 128], F32, tag="eb")
                        _gather(nc, eb[:], ec_full[:], pb_i[:, hh:hh + 1])
                        bT_p = psp.tile([128, 128], F32, tag="tT")
                        nc.tensor.transpose(out=bT_p[:], in_=eb[:],
                                            identity=ident[:])
                        bT = pool.tile([128, 128], F32, tag="tTs")
                        nc.scalar.copy(bT[:], bT_p[:])
                        nc.tensor.matmul(out=yac[:, half * 128:(half + 1) * 128],
                                         lhsT=w1b[:], rhs=bT[:],
                                         start=False, stop=True)
                    y0 = pool.tile([128, 256], F32, tag="y0")
                    nc.scalar.activation(out=y0[:], in_=yac[:], func=ACT.Identity,
                                         bias=b1s[:])
                    ys = pool.tile([128, 256], F32, tag="ys")
                    nc.scalar.mul(ys[:], y0[:], 0.01)
                    y = pool.tile([128, 256], F32, tag="y")
                    nc.vector.tensor_tensor(out=y[:], in0=y0[:], in1=ys[:],
                                            op=ALU.max)
                    dl = psp.tile([1, 256], F32, tag="dl", bufs=1)
                    nc.tensor.matmul(out=dl[:], lhsT=w2d[:, 0:1], rhs=y[:],
                                     start=True, stop=True)
                    p0 = pool.tile([1, 256], F32, tag="p0")
                    nc.scalar.activation(out=p0[:], in_=dl[:], func=ACT.Sigmoid,
                                         bias=b2s[:], scale=1.0)
                    nb2 = pool.tile([1, 1], F32, tag="nb2")
                    nc.scalar.mul(nb2[:], b2s[:], -1.0)
                    p1 = pool.tile([1, 256], F32, tag="p1")
                    nc.scalar.activation(out=p1[:], in_=dl[:], func=ACT.Sigmoid,
                                         bias=nb2[:], scale=-1.0)
                    nc.sync.dma_start(out=pout[0:1, ch * 256:(ch + 1) * 256],
                                      in_=p0[:])
                    nc.sync.dma_start(out=pout[1:2, ch * 256:(ch + 1) * 256],
                                      in_=p1[:])
    nc.compile()
    return nc


def preprocess(batch, int_node_ids, int_neigh_ids, ext_neigh,
               E, W, M, U, V, W1, b1, W2, b2, ncores=NC_):
    """Build per-core input maps (index reordering + weight replication)."""
    nsh = N // ncores
    NP = B // ncores
    ids = np.asarray(int_node_ids).astype(np.int32)
    idn = np.asarray(int_neigh_ids).astype(np.int32)
    ext = np.asarray(ext_neigh).astype(np.int64)
    bat = np.asarray(batch).astype(np.int64)
    # h_full row for global node n (3-chunk split AllGather: each chunk's
    # per-core slices land consecutively)
    own_e = ext // nsh
    loc_e = ext % nsh
    extg = np.where(
        loc_e < H1, own_e * H1 + loc_e,
        np.where(loc_e < H2,
                 ncores * H1 + own_e * (H2 - H1) + (loc_e - H1),
                 ncores * H2 + own_e * (NS - H2) + (loc_e - H2))
    ).astype(np.int32)
    # ec_full slot for each pair endpoint: per-owner compacted unique rows
    allp = np.concatenate([bat[:, 0], bat[:, 1]])
    slot_of = np.full(N, -1, np.int64)
    cmp_lists = []
    for c in range(ncores):
        u = np.unique(allp[allp // nsh == c] % nsh)
        if len(u) > CMP:
            raise ValueError(f"core {c} needs {len(u)} > {CMP} pair rows")
        slot_of[c * nsh + u] = c * CMP + np.arange(len(u))
        lu = np.zeros(CMP, np.int32)
        lu[:len(u)] = u
        cmp_lists.append(lu)
    bga = slot_of[bat[:, 0]].astype(np.int32)
    bgb = slot_of[bat[:, 1]].astype(np.int32)
    assert bga.min() >= 0 and bgb.min() >= 0

    Ef = np.ascontiguousarray(np.asarray(E, np.float32))
    w = {
        "WT": np.ascontiguousarray(np.asarray(W, np.float32).T),
        "MT": np.ascontiguousarray(np.asarray(M, np.float32).T),
        "UT": np.ascontiguousarray(np.asarray(U, np.float32).T),
        "VT": np.ascontiguousarray(np.asarray(V, np.float32).T),
        "W1aT": np.ascontiguousarray(np.asarray(W1, np.float32)[:, :128].T),
        "W1bT": np.ascontiguousarray(np.asarray(W1, np.float32)[:, 128:].T),
        "b1t": np.asarray(b1, np.float32).reshape(128, 1),
        "w2dT": np.ascontiguousarray(
            (np.asarray(W2, np.float32)[0] - np.asarray(W2, np.float32)[1])
            .reshape(128, 1)),
        "b2d": np.array([[np.float32(b2[0]) - np.float32(b2[1])]], np.float32),
    }
    in_maps = []
    for c in range(ncores):
        lo = c * nsh
        ids_p = np.zeros((NS, K), np.int32)
        idn_p = np.zeros((NS, K, J), np.int32)
        ext_p = np.zeros((NS, NEXT), np.int32)
        ids_p[:nsh] = ids[lo:lo + nsh]
        idn_p[:nsh] = idn[lo:lo + nsh]
        ext_p[:nsh] = extg[lo:lo + nsh]
        embI = np.ascontiguousarray(ids_p.reshape(NT, 128).T)
        nbrI = np.ascontiguousarray(
            idn_p.reshape(NT, 128, J).transpose(1, 0, 2).reshape(128, NT * J))
        extI = np.ascontiguousarray(
            ext_p.reshape(NB, 128, NEXT).transpose(1, 0, 2)
            .reshape(128, NB * NEXT))
        pAIc = np.ascontiguousarray(
            bga[c * NP:(c + 1) * NP].reshape(NP // 128, 128).T)
        pBIc = np.ascontiguousarray(
            bgb[c * NP:(c + 1) * NP].reshape(NP // 128, 128).T)
        cmpIc = np.ascontiguousarray(
            cmp_lists[c].reshape(CMP // 128, 128).T)
        in_maps.append({"E": Ef, "nbrI": nbrI, "embI": embI, "extI": extI,
                        "pAI": pAIc, "pBI": pBIc, "cmpI": cmpIc, **w})
    return in_maps


def postprocess(results, ncores=NC_):
    NP = B // ncores
    out = np.zeros((B, 2), np.float32)
    for c in range(ncores):
        p = results[c]["pout"]              # [2, NP]
        out[c * NP:(c + 1) * NP, 0] = p[0]
        out[c * NP:(c + 1) * NP, 1] = p[1]
    return out


_NC_CACHE = None


def kernel(batch, int_node_ids, int_neigh_ids, ext_neigh,
           E, W, M, U, V, W1, b1, W2, b2):
    global _NC_CACHE
    in_maps = preprocess(batch, int_node_ids, int_neigh_ids, ext_neigh,
                         E, W, M, U, V, W1, b1, W2, b2)
    if _NC_CACHE is None:
        _NC_CACHE = build()
    res = run_bass_kernel_spmd(_NC_CACHE, in_maps, core_ids=list(range(NC_)))
    return postprocess(res.results)

